# revision 10
# baseline (speedup 1.0000x reference)
"""FPN ROI-align (crop + bilinear + 2x2 maxpool) Trainium2 kernel.

Problem: p2..p5 FPN pyramid [1,256,S,S] (S=256,128,64,32), rois [1000,5]
-> out [1000, 256, 7, 7] float32.

Strategy (8 NeuronCores, SPMD):
  - Shard ROIs: 125 per core; replicate the features.
  - Host builds a stacked-pair NHWC table T2: row r = [pixel r | pixel
    below r] (level-aware, edge-clamped), 512 floats per row.  One
    indirect-DMA gather row-pair (1024 floats starting at row ilo) then
    contains all four bilinear corners of a sample:
      [T(ylo,bx), T(ybelow,bx), T(ylo,bx+1), T(ybelow,bx+1)].
  - Host precomputes per output point and per 2x2-pool sample plane the
    gather index ilo and four folded weights (bilinear x bounds masks,
    edge-degenerate rows folded onto the top chunk).
  - Device: per batch of 128 points and per plane, one
    nc.gpsimd.indirect_dma_start gather [128 descriptors x 4KB],
    per-partition-scalar blends (ACT scale-copy + DVE
    scalar_tensor_tensor), elementwise max across the four planes,
    contiguous DMA out.
"""
import os
import sys

import numpy as np

for _p in ("/opt/trn_rl_repo", "/root/.axon_site/_ro/trn_rl_repo"):
    if _p not in sys.path and os.path.isdir(_p):
        sys.path.append(_p)

import bass_rust  # noqa: E402
from concourse import bass, mybir  # noqa: E402
import concourse.tile as tile  # noqa: E402
from concourse.bass_utils import run_bass_kernel_spmd  # noqa: E402
from concourse.vector_clock import ScopedClock  # noqa: E402

_MAX_WAITS = 1
_NOP_SEQ = [0]


def _patched_add_instruction(self, inst):
    """Wrap TileContext._add_instruction: the pinned walrus codegen allows
    at most one sync wait per instruction, so hoist excess waits onto
    single-wait NOPs queued just before on the same engine."""
    si = inst.sync_info
    if si is not None and len(si.on_wait) > _MAX_WAITS:
        waits = list(si.on_wait)
        extra, keep = waits[:-_MAX_WAITS], waits[-_MAX_WAITS:]
        for w in extra:
            _NOP_SEQ[0] += 1
            nop = bass_rust.InstNoOp(name=f"wsplit-{_NOP_SEQ[0]}", engine=inst.engine)
            nop.sync_info = bass_rust.SyncInfo(on_wait=[w], on_update=[])
            nop.bass_nofuse = True
            _orig_add_instruction(self, nop)
        inst.sync_info = bass_rust.SyncInfo(
            on_wait=keep, on_update=list(si.on_update)
        )
    _orig_add_instruction(self, inst)


_orig_add_instruction = tile.TileContext._add_instruction
if getattr(tile.TileContext, "_wsplit_patched", False):
    _orig_add_instruction = tile.TileContext._wsplit_orig
tile.TileContext._add_instruction = _patched_add_instruction
tile.TileContext._wsplit_patched = True
tile.TileContext._wsplit_orig = _orig_add_instruction


def _split_wait_drain_and_barrier(self, tick_clock, wait_clock):
    """Replacement for TileContext._drain_and_barrier (same wait limit)."""
    nc = self.nc
    probe = nc.sync.nop(nofuse=True)
    wait_clock.add_sem_waits(
        probe.ins, ScopedClock({None: tick_clock.global_clock})
    )
    si = probe.ins.sync_info
    waits = list(si.on_wait) if si is not None else []
    if si is not None:
        probe.ins.sync_info = bass_rust.SyncInfo(on_wait=waits[:1], on_update=[])
    for w in waits[1:]:
        n = nc.sync.nop(nofuse=True)
        n.ins.sync_info = bass_rust.SyncInfo(on_wait=[w], on_update=[])
    nc.sync.drain()

    nc.all_engine_barrier()
    assert self.sems is not None
    popped = nc._tile_sem_poison_stack.pop()
    assert popped is self._sem_poison
    nc.clear_and_free_semaphores(list(self.sems.allocated().values()))
    nc.all_engine_barrier()


tile.TileContext._drain_and_barrier = _split_wait_drain_and_barrier

# ---------------------------------------------------------------- constants
POOL = 7
PRE = 14
C = 256
C2 = 2 * C
N_ROIS = 1000
N_CORES = 8
ROIS_PER_CORE = N_ROIS // N_CORES          # 125
NPTS = ROIS_PER_CORE * POOL * POOL         # 6125 output points per core
NB = (NPTS + 127) // 128                   # 48 batches of 128 points
LEVEL_HW = np.array([256, 128, 64, 32], np.int64)
BASES = np.array([0, 65536, 81920, 86016], np.int64)
R_TAB = 87040 + 2                          # T2 rows incl. 2 zero pad rows

TABLE_NP_DT = np.float16
TABLE_MB_DT = mybir.dt.float16
BLEND_NP_DT = np.float16
BLEND_MB_DT = mybir.dt.float16

# ---------------------------------------------------------------- host math


def _build_t2(p2, p3, p4, p5):
    """Stacked-pair table [R_TAB, 512]: row (lvl,y,x) = [T(y,x) | T(y+1c,x)]."""
    parts = []
    for p in (p2, p3, p4, p5):
        L = np.transpose(p[0], (1, 2, 0))          # [H, W, C]
        H = L.shape[0]
        below = L[np.minimum(np.arange(H) + 1, H - 1)]
        parts.append(np.concatenate([L, below], axis=-1).reshape(-1, C2))
    parts.append(np.zeros((2, C2), np.float32))
    return np.ascontiguousarray(np.concatenate(parts, axis=0).astype(TABLE_NP_DT))


def _roi_sample_data(rois):
    """f32-faithful mirror of the reference's coordinate math.

    Returns ilo [N,14,14] int64 (T2 gather row) and chunk weights
    WA,WB,WC,WD [N,14,14] f32 for the gathered chunks
    [top(bx), below(bx), top(bx+1), below(bx+1)].
    """
    f32 = np.float32
    x1 = rois[:, 1].astype(f32)
    y1 = rois[:, 2].astype(f32)
    x2 = rois[:, 3].astype(f32)
    y2 = rois[:, 4].astype(f32)
    w = np.where(x2 - x1 <= 0, f32(1e-14), x2 - x1).astype(f32)
    h = np.where(y2 - y1 <= 0, f32(1e-14), y2 - y1).astype(f32)
    kf = f32(4.0) + np.log2(np.sqrt(w * h) / f32(224.0)).astype(f32)
    kf = np.clip(kf, f32(2.0), f32(5.0))
    k = np.round(kf)
    scale = np.exp2(k).astype(f32)
    lvl = k.astype(np.int64) - 2
    Wl = LEVEL_HW[lvl]
    x1s, y1s, x2s, y2s = x1 / scale, y1 / scale, x2 / scale, y2 / scale

    t = np.linspace(-1.0, 1.0, PRE, dtype=f32)
    px = (x1s + x2s)[:, None] * f32(0.5) + t[None, :] * ((x2s - x1s)[:, None] * f32(0.5))
    py = (y1s + y2s)[:, None] * f32(0.5) + t[None, :] * ((y2s - y1s)[:, None] * f32(0.5))

    # x axis: pair base bx in [0, Wl-2]; per-column weights cw0, cw1
    u0 = np.floor(px)
    dx = (px - u0).astype(f32)
    u0i = u0.astype(np.int64)
    in_u0 = (u0i >= 0) & (u0i < Wl[:, None])
    in_u1 = (u0i + 1 >= 0) & (u0i + 1 < Wl[:, None])
    a0 = (f32(1.0) - dx) * in_u0
    a1 = dx * in_u1
    bx = np.clip(u0i, 0, (Wl - 2)[:, None])
    cw0 = a0 * (u0i == bx) + a1 * (u0i + 1 == bx)
    cw1 = a0 * (u0i == bx + 1) + a1 * (u0i + 1 == bx + 1)

    # y axis: top row ylo in [0, Wl-1]; top/bottom weights (edge-degenerate
    # bottom rows folded onto top, since T2's below-row is edge-clamped)
    v0 = np.floor(py)
    dy = (py - v0).astype(f32)
    v0i = v0.astype(np.int64)
    b0 = (f32(1.0) - dy) * ((v0i >= 0) & (v0i < Wl[:, None]))
    b1 = dy * ((v0i + 1 >= 0) & (v0i + 1 < Wl[:, None]))
    ylo = np.clip(v0i, 0, (Wl - 1)[:, None])
    yhi = np.clip(v0i + 1, 0, (Wl - 1)[:, None])
    same = yhi == ylo
    w_top = b0 + b1 * same
    w_bot = b1 * (~same)

    base = BASES[lvl]
    ilo = base[:, None, None] + ylo[:, :, None] * Wl[:, None, None] + bx[:, None, :]
    WA = w_top[:, :, None] * cw0[:, None, :]
    WB = w_bot[:, :, None] * cw0[:, None, :]
    WC = w_top[:, :, None] * cw1[:, None, :]
    WD = w_bot[:, :, None] * cw1[:, None, :]
    return ilo, WA.astype(f32), WB.astype(f32), WC.astype(f32), WD.astype(f32)


def _pack_core(rois_chunk):
    """Pack one core's gather indices / weights.

    Point pid = roi_local*49 + oy*7 + ox lives at batch b = pid//128,
    partition p = pid%128.  Plane q = a*2+bb is sample (2oy+a, 2ox+bb).

    Returns idx [4, 128, NB] int32, wts [4, 128, NB*4] float32.
    """
    N = rois_chunk.shape[0]
    ilo, WA, WB, WC, WD = _roi_sample_data(rois_chunk)

    idx_out = np.zeros((4, 128, NB), np.int32)
    wts_out = np.zeros((4, 128, NB * 4), np.float32)

    oy, ox = np.meshgrid(np.arange(POOL), np.arange(POOL), indexing="ij")
    pid = (np.arange(N)[:, None, None] * 49 + (oy * 7 + ox)[None]).reshape(-1)
    bq = pid // 128
    pq = pid % 128
    nn = np.repeat(np.arange(N), 49)
    for q in range(4):
        a, bb = q // 2, q % 2
        iy = np.tile((2 * oy + a).reshape(-1), N)
        ix = np.tile((2 * ox + bb).reshape(-1), N)
        sel = (nn, iy, ix)
        idx_out[q, pq, bq] = ilo[sel].astype(np.int32)
        wts_out[q, pq, bq * 4 + 0] = WA[sel]
        wts_out[q, pq, bq * 4 + 1] = WB[sel]
        wts_out[q, pq, bq * 4 + 2] = WC[sel]
        wts_out[q, pq, bq * 4 + 3] = WD[sel]
    return idx_out, wts_out


# ---------------------------------------------------------------- device program

_NC_CACHE = None


def build_program():
    global _NC_CACHE
    if _NC_CACHE is not None:
        return _NC_CACHE
    f32 = mybir.dt.float32
    fb = BLEND_MB_DT
    nc = bass.Bass(num_swdge_queues=4)
    t2 = nc.declare_dram_parameter("t2", [R_TAB, C2], TABLE_MB_DT, isOutput=False)
    idx_p = nc.declare_dram_parameter("idx", [4, 128, NB], mybir.dt.int32, isOutput=False)
    wts_p = nc.declare_dram_parameter("wts", [4, 128, NB * 4], f32, isOutput=False)
    out_p = nc.declare_dram_parameter("out", [NB // 2, 128, 2 * C], fb, isOutput=True)

    Copy = mybir.ActivationFunctionType.Copy
    add = mybir.AluOpType.add
    mult = mybir.AluOpType.mult
    amax = mybir.AluOpType.max

    with tile.TileContext(nc) as tc:
        with (
            tc.tile_pool(name="const", bufs=1) as cpool,
            tc.tile_pool(name="gp", bufs=8) as gpool,
            tc.tile_pool(name="bp", bufs=8) as bpool,
            tc.tile_pool(name="tp", bufs=10) as tpool,
            tc.tile_pool(name="op", bufs=3) as opool,
        ):
            idx_t = []
            wts_t = []
            for q in range(4):
                it = cpool.tile([128, NB], mybir.dt.int32, tag=f"idx{q}")
                wt = cpool.tile([128, NB * 4], f32, tag=f"wts{q}")
                nc.sync.dma_start(out=it[:], in_=idx_p[q])
                nc.sync.dma_start(out=wt[:], in_=wts_p[q])
                idx_t.append(it)
                wts_t.append(wt)

            dma_i = [0]

            def gather(q, b, g):
                inst = nc.gpsimd.indirect_dma_start(
                    out=g[:],
                    out_offset=None,
                    in_=t2[:],
                    in_offset=bass.IndirectOffsetOnAxis(
                        ap=idx_t[q][:, b : b + 1], axis=0
                    ),
                )
                qn = dma_i[0] % 4
                dma_i[0] += 1
                if qn:
                    inst.ins.queue = f"qPoolDynamic{qn}"
                return inst

            for bb in range(NB // 2):
                btiles = []
                for q in range(4):
                    bt = bpool.tile([128, 512], fb, tag="bt")
                    for half in range(2):
                        b = 2 * bb + half
                        g = gpool.tile([128, 4 * C], TABLE_MB_DT, tag="g")
                        gather(q, b, g)
                        # chunks: [top(bx), below(bx), top(bx+1), below(bx+1)]
                        g0 = g[:, 0:256]
                        g1 = g[:, 256:512]
                        g2 = g[:, 512:768]
                        g3 = g[:, 768:1024]
                        wcol = b * 4
                        wv = lambda kk: wts_t[q][:, wcol + kk : wcol + kk + 1]
                        t1 = tpool.tile([128, 256], fb, tag="tmp")
                        t2w = tpool.tile([128, 256], fb, tag="tmp")
                        nc.scalar.activation(t1[:], g0, Copy, scale=wv(0))
                        nc.scalar.activation(t2w[:], g1, Copy, scale=wv(1))
                        nc.vector.tensor_tensor(t1[:], t1[:], t2w[:], add)
                        nc.vector.scalar_tensor_tensor(t2w[:], g2, wv(2), t1[:], mult, add)
                        nc.vector.scalar_tensor_tensor(
                            bt[:, half * 256 : (half + 1) * 256], g3, wv(3), t2w[:],
                            mult, add,
                        )
                    btiles.append(bt)

                m01 = tpool.tile([128, 512], fb, tag="tmp2")
                m23 = tpool.tile([128, 512], fb, tag="tmp2")
                ot = opool.tile([128, 512], fb, tag="o")
                nc.vector.tensor_tensor(m01[:], btiles[0][:], btiles[1][:], amax)
                nc.vector.tensor_tensor(m23[:], btiles[2][:], btiles[3][:], amax)
                nc.vector.tensor_tensor(ot[:], m01[:], m23[:], amax)
                nc.sync.dma_start(out=out_p[bb], in_=ot[:])

    _NC_CACHE = nc
    return nc


# ---------------------------------------------------------------- entry point


def kernel(p2, p3, p4, p5, rois, **run_kwargs):
    p2, p3, p4, p5, rois = (
        np.asarray(p2), np.asarray(p3), np.asarray(p4), np.asarray(p5),
        np.asarray(rois),
    )
    nc = build_program()
    t2 = _build_t2(p2, p3, p4, p5)
    in_maps = []
    for core in range(N_CORES):
        chunk = rois[core * ROIS_PER_CORE : (core + 1) * ROIS_PER_CORE]
        idx, wts = _pack_core(chunk)
        in_maps.append({"t2": t2, "idx": idx, "wts": wts})
    res = run_bass_kernel_spmd(nc, in_maps, core_ids=list(range(N_CORES)), **run_kwargs)

    outs = []
    for core in range(N_CORES):
        flat = (
            np.asarray(res.results[core]["out"])
            .astype(np.float32)
            .reshape(NB // 2, 128, 2, C)
            .transpose(0, 2, 1, 3)
            .reshape(-1, C)[:NPTS]
        )
        outs.append(
            flat.reshape(ROIS_PER_CORE, POOL, POOL, C).transpose(0, 3, 1, 2)
        )
    out = np.ascontiguousarray(np.concatenate(outs, axis=0))
    if run_kwargs:
        return out, res
    return out
